# revision 6
# baseline (speedup 1.0000x reference)
"""Dilated (LongNet-style) attention kernel for 8 TRN2 NeuronCores — v2.

Same head-sharded single-AllToAll strategy as v1, rebuilt around three
trace-driven fixes:
  - inputs stream as bf16 in 2MB quarter-chunks on HWDGE queues only
    (24MB instead of 48MB; ~400GB/s instead of ~190; gpsimd freed)
  - all on-chip activations (Q^T/K^T/V^T, e, vb) are bf16: halves SBUF
    and doubles most DVE copy throughput
  - emission keeps the PE stream dense (proj quarters and attention jobs
    interleaved without DMA waits between matmul bursts) so the HAM
    clock-gate stays at 8/8 instead of the 4/8 the v1 trace showed.
"""

import sys

if "/opt/trn_rl_repo" not in sys.path:
    sys.path.insert(0, "/opt/trn_rl_repo")

import numpy as np

import concourse.bacc as bacc
import concourse.bass as bass  # noqa: F401
import concourse.mybir as mybir
import concourse.tile as tile
from concourse import bass_utils

F32 = mybir.dt.float32
F32R = mybir.dt.float32r
BF16 = mybir.dt.bfloat16
AF = mybir.ActivationFunctionType

N_CORES = 8
E, L, H, D = 1024, 4096, 16, 64
KC = 8          # contraction chunks of 128 for the projections
PB = 512        # position block
NPB = L // PB   # 8
QTR = 1024      # dma quarter: positions per input-stream chunk
G = 1024        # compressed segment length (same for every branch)
VBW = 65        # V_both per-chunk width (64 feats + ones col)


def _build(stage=6, dbg=False):
    nc = bacc.Bacc("TRN2", target_bir_lowering=False, debug=False,
                   num_devices=N_CORES)

    # inputs: per-stream packed [pos-block, partition(=feat%128), kc*512pos]
    xq = nc.dram_tensor("xq", [8, 128, KC * PB], BF16, kind="ExternalInput")
    xk = nc.dram_tensor("xk", [8, 128, KC * PB], BF16, kind="ExternalInput")
    xv = nc.dram_tensor("xv", [8, 128, KC * PB], BF16, kind="ExternalInput")
    wq = nc.dram_tensor("wq", [128, KC * 128], BF16, kind="ExternalInput")
    wk = nc.dram_tensor("wk", [128, KC * 128], BF16, kind="ExternalInput")
    wv = nc.dram_tensor("wv", [128, KC * 128], BF16, kind="ExternalInput")
    wo = nc.dram_tensor("wo", [2, 128, 4 * E], BF16, kind="ExternalInput")
    bq = nc.dram_tensor("bq", [128, 1], F32, kind="ExternalInput")
    bk = nc.dram_tensor("bk", [128, 1], F32, kind="ExternalInput")
    bo8 = nc.dram_tensor("bo8", [128, 8], F32, kind="ExternalInput")
    ind2 = nc.dram_tensor("ind2", [2, 128], F32R, kind="ExternalInput")
    eyer = nc.dram_tensor("eyer", [128, 128], BF16, kind="ExternalInput")
    ones16 = nc.dram_tensor("ones16", [128, 16], BF16, kind="ExternalInput")
    wsel = nc.dram_tensor("wsel", [128, 2], F32, kind="ExternalInput")

    outT = nc.dram_tensor("outT", [E, PB], F32, kind="ExternalOutput")
    if dbg:
        dbg_qt = nc.dram_tensor("dbg_qt", [128, L], BF16, kind="ExternalOutput")
        dbg_kt = nc.dram_tensor("dbg_kt", [128, L], BF16, kind="ExternalOutput")
        dbg_vt = nc.dram_tensor("dbg_vt", [128, L], BF16, kind="ExternalOutput")
        dbg_q2 = nc.dram_tensor("dbg_q2", [128, G], BF16, kind="ExternalOutput")
        dbg_acc = nc.dram_tensor("dbg_acc", [128, L], F32, kind="ExternalOutput")
        dbg_zz = nc.dram_tensor("dbg_zz", [65, L], F32, kind="ExternalOutput")

    a2a_warm_in = nc.dram_tensor("a2a_warm_in", [8, 1, 64], F32R)
    a2a_warm_out = nc.dram_tensor("a2a_warm_out", [8, 1, 64], F32R)
    a2a_in = nc.dram_tensor("a2a_in", [8, 128, PB], BF16)
    a2a_out = nc.dram_tensor("a2a_out", [8, 128, PB], BF16)

    import contextlib

    def _emit(tc, ctx):
        pin = ctx.enter_context(tc.tile_pool(name="pin", bufs=6))
        persist = ctx.enter_context(tc.tile_pool(name="persist", bufs=1))
        vpool = ctx.enter_context(tc.tile_pool(name="vpool", bufs=3))
        epool = ctx.enter_context(tc.tile_pool(name="epool", bufs=4))
        opool = ctx.enter_context(tc.tile_pool(name="opool", bufs=3))
        psw = ctx.enter_context(tc.tile_pool(name="psw", bufs=2, space="PSUM"))
        pso = ctx.enter_context(tc.tile_pool(name="pso", bufs=2, space="PSUM"))

        # ---- small constants (scalar queue; inputs go on sync) ----
        wq_sb = persist.tile([128, KC * 128], BF16, tag="wq")
        wk_sb = persist.tile([128, KC * 128], BF16, tag="wk")
        wv_sb = persist.tile([128, KC * 128], BF16, tag="wv")
        nc.scalar.dma_start(wq_sb[:], wq[:])
        nc.scalar.dma_start(wk_sb[:], wk[:])
        nc.scalar.dma_start(wv_sb[:], wv[:])
        bq_sb = persist.tile([128, 1], F32, tag="bq")
        bk_sb = persist.tile([128, 1], F32, tag="bk")
        bo_sb = persist.tile([128, 8], F32, tag="bo")
        ind_sb = persist.tile([2, 128], F32R, tag="ind")
        eye_sb = persist.tile([128, 128], BF16, tag="eye")
        on_sb = persist.tile([128, 16], BF16, tag="on")
        ws_sb = persist.tile([128, 2], F32, tag="ws")
        nc.scalar.dma_start(bq_sb[:], bq[:])
        nc.scalar.dma_start(bk_sb[:], bk[:])
        nc.scalar.dma_start(bo_sb[:], bo8[:])
        nc.scalar.dma_start(ind_sb[:], ind2[:])
        nc.scalar.dma_start(eye_sb[:], eyer[:])
        nc.scalar.dma_start(on_sb[:], ones16[:])
        nc.scalar.dma_start(ws_sb[:], wsel[:])

        QT = persist.tile([128, L], BF16, tag="QT")
        KT = persist.tile([128, L], BF16, tag="KT")
        VT = persist.tile([128, L], BF16, tag="VT")
        QT2 = persist.tile([128, G], BF16, tag="QT2")
        KT2 = persist.tile([128, G], BF16, tag="KT2")
        VT2 = persist.tile([128, G], BF16, tag="VT2")
        acc = persist.tile([128, L], F32, tag="acc")
        zz = persist.tile([65, L], F32, tag="zz")
        zst = persist.tile([65, G], F32, tag="zst")  # rows 0/64 (lane-aligned)
        mg = persist.tile([128, 8 * PB], BF16, tag="mg")
        wo_sb_0 = persist.tile([128, 4 * E], BF16, tag="wo0")
        wo_sb_1 = persist.tile([128, 4 * E], BF16, tag="wo1")

        streams = {
            "k": (xk, wk_sb, KT, bk_sb),
            "v": (xv, wv_sb, VT, None),
            "q": (xq, wq_sb, QT, bq_sb),
        }

        xin_tiles = {}

        def load_pb(name, pb):
            x_d = streams[name][0]
            xin = pin.tile([128, KC * PB], BF16, tag="xin")
            nc.sync.dma_start(xin[:], x_d[pb])
            xin_tiles[(name, pb)] = xin

        def load_quarter(name, qt):
            load_pb(name, 2 * qt)
            load_pb(name, 2 * qt + 1)

        def proj_pb(name, pb):
            """Project one 1MB input position block."""
            x_d, w_t, dst, bias = streams[name]
            xin = xin_tiles.pop((name, pb))
            pt = psw.tile([128, 1024], F32, tag="w")
            for kc in range(KC):
                nc.tensor.matmul(
                    pt[:, 0:PB],
                    w_t[:, kc * 128:(kc + 1) * 128],
                    xin[:, kc * PB:(kc + 1) * PB],
                    start=(kc == 0), stop=(kc == KC - 1),
                )
            dslice = dst[:, pb * PB:(pb + 1) * PB]
            if name == "k":
                if pb < 2:
                    nc.scalar.add(dslice, pt[:, 0:PB], bk_sb[:, 0:1])
                else:
                    nc.vector.tensor_scalar_add(dslice, pt[:, 0:PB],
                                                bk_sb[:, 0:1])
            elif name == "q":
                nc.vector.tensor_scalar_add(dslice, pt[:, 0:PB], bias[:])
            else:
                nc.vector.tensor_copy(dslice, pt[:, 0:PB])

        def proj_quarter(name, qt):
            proj_pb(name, 2 * qt)
            proj_pb(name, 2 * qt + 1)

        def b2_copies():
            # slot A picks dense offset 0 or 1, slot B picks 2 or 3, via
            # 0/1 indicators in ws_sb (core-uniform instruction stream).
            for src, dst, eng in ((QT, QT2, nc.vector), (KT, KT2, nc.vector),
                                  (VT, VT2, nc.vector)):
                for slot in range(2):
                    p0, p1 = 64 * slot, 64 * slot + 64
                    o0 = 2 * slot
                    eng.tensor_scalar_mul(
                        dst[p0:p1, :], src[p0:p1, o0::4], ws_sb[p0:p1, 0:1]
                    )
                    eng.scalar_tensor_tensor(
                        dst[p0:p1, :], src[p0:p1, o0 + 1::4],
                        ws_sb[p0:p1, 1:2], dst[p0:p1, :],
                        mybir.AluOpType.mult, mybir.AluOpType.add,
                    )

        def kq_slice(br, seg, slot, t, lo, size):
            if br == 0:
                base = 1024 * seg + lo
                return t[slot * 64:(slot + 1) * 64, base:base + size]
            if br == 1:
                base = 2048 * seg + 2 * lo + slot
                return t[slot * 64:(slot + 1) * 64,
                         base:base + 2 * size - slot:2]
            return t[slot * 64:(slot + 1) * 64, lo:lo + size]

        def mm_ranges(jc):
            if jc <= 3:
                return [(128 * jc, 512 - 128 * jc), (512, 512)]
            return [(128 * jc, 1024 - 128 * jc)]

        def vprep(br, seg, vb, act_b=True):
            """V_both prep (PE transpose -> vb, bf16). Copies split DVE
            (slot A) / ACT (slot B: only when act_b — during the late,
            exp-saturated jobs everything stays on DVE)."""
            nc.vector.tensor_copy(vb[:, 64::VBW], on_sb[:])
            for jc in range(8):
                if br == 0:
                    tp = psw.tile([128, 1024], BF16, tag="w")
                    src = VT[:, 1024 * seg + 128 * jc:1024 * seg + 128 * (jc + 1)]
                    nc.tensor.transpose(tp[:, 0:128], src, eye_sb[:])
                    nc.vector.tensor_copy(
                        vb[:, jc * VBW:jc * VBW + 64], tp[:, 0:64])
                    bcp = nc.scalar.copy if act_b else nc.vector.tensor_copy
                    bcp(vb[:, 8 * VBW + jc * VBW:8 * VBW + jc * VBW + 64],
                        tp[:, 64:128])
                else:
                    for slot in range(2):
                        tp = psw.tile([128, 1024], BF16, tag="w")
                        if br == 1:
                            base = 2048 * seg + 256 * jc + slot
                            src = VT[slot * 64:(slot + 1) * 64,
                                     base:base + 256 - slot:2]
                        else:
                            src = VT2[slot * 64:(slot + 1) * 64,
                                      128 * jc:128 * (jc + 1)]
                        nc.tensor.transpose(
                            tp[:, 0:64], src,
                            eye_sb[slot * 64:(slot + 1) * 64,
                                   slot * 64:(slot + 1) * 64],
                        )
                        if slot == 0:
                            nc.vector.tensor_copy(
                                vb[:, jc * VBW:jc * VBW + 64], tp[:, 0:64])
                        elif act_b:
                            nc.scalar.copy(
                                vb[:, 8 * VBW + jc * VBW:
                                   8 * VBW + jc * VBW + 64],
                                tp[:, 0:64])
                        else:
                            nc.vector.tensor_copy(
                                vb[:, 8 * VBW + jc * VBW:
                                   8 * VBW + jc * VBW + 64],
                                tp[:, 0:64])

        def job(br, seg, inject=(), fill=(), vb=None):
            """One (branch, segment) attention job, software-pipelined:
            scores(jc+1) is emitted before AV(jc) so the PE never waits on
            the exp chain. `inject` closures (deferred merges / norm prep)
            are emitted between the first scores and the steady loop.
            `fill` closures (proj position-blocks) are spread one per jc
            iteration as PE filler so the engine stays dense (HAM-warm)
            while the ACT exp chain runs."""
            fill = list(fill)
            kt_src = KT2 if br == 2 else KT
            qt_src = QT2 if br == 2 else QT

            o_ps_a = pso.tile([128, 1024], F32, tag="o")
            o_ps_b = pso.tile([128, 1024], F32, tag="o")
            o_ps = [o_ps_a, o_ps_b]

            def scores(jc):
                s_ps_a = psw.tile([128, 1024], F32, tag="w")
                s_ps_b = psw.tile([128, 1024], F32, tag="w")
                s_ps = [s_ps_a, s_ps_b]
                for slot in range(2):
                    for lo, size in mm_ranges(jc):
                        lhs = kq_slice(br, seg, slot, kt_src, 128 * jc, 128)
                        rhs = kq_slice(br, seg, slot, qt_src, lo, size)
                        nc.tensor.matmul(
                            s_ps[slot][:, lo:lo + size], lhs, rhs,
                            start=True, stop=True,
                            tile_position=(slot * 64, 0),
                        )
                return s_ps

            s_cur = scores(0)
            if vb is None:
                vb = vpool.tile([128, 2 * 8 * VBW], BF16, tag="vb")
                vprep(br, seg, vb)
            for cl in inject:
                cl()
            for jc in range(8):
                e_t = [None, None]
                for slot in range(2):
                    c0 = 128 * jc
                    e = epool.tile([128, 1024], BF16, tag="e")  # noqa
                    nc.scalar.activation(
                        e[:, c0:1024], s_cur[slot][:, c0:1024], AF.Exp
                    )
                    nc.gpsimd.affine_select(
                        e[:, c0:c0 + 128], e[:, c0:c0 + 128],
                        pattern=[[1, 128]],
                        compare_op=mybir.AluOpType.is_ge,
                        fill=0.0, base=0, channel_multiplier=-1,
                    )
                    e_t[slot] = e
                if jc < 7:
                    if fill:
                        fill.pop(0)()
                    s_cur = scores(jc + 1)
                for slot in range(2):
                    for lo, size in mm_ranges(jc):
                        nc.tensor.matmul(
                            o_ps[slot][0:VBW, lo:lo + size],
                            vb[:, slot * 8 * VBW + jc * VBW:
                               slot * 8 * VBW + (jc + 1) * VBW],
                            e_t[slot][:, lo:lo + size],
                            start=(jc == 0), stop=(jc == 7),
                        )

            def merge():
                _merge(br, seg, o_ps)
            return merge

        def _merge(br, seg, o_ps):
            # merge into acc / zz (slot B copies on ACT to offload DVE).
            # zz updates go first so the normalization reciprocal chain
            # can start as early as possible.
            if br == 0:
                sl_ = slice(1024 * seg, 1024 * (seg + 1))
                nc.vector.tensor_copy(zz[0:1, sl_], o_ps[0][64:65, :])
                nc.scalar.copy(zz[64:65, sl_], o_ps[1][64:65, :])
                nc.vector.tensor_copy(acc[0:64, sl_], o_ps[0][0:64, :])
                nc.scalar.copy(acc[64:128, sl_], o_ps[1][0:64, :])
            elif br == 1:
                for slot in range(2):
                    zc = zz[64 * slot:64 * slot + 1,
                            2048 * seg + slot:2048 * (seg + 1):2]
                    nc.vector.tensor_add(zc, zc, o_ps[slot][64:65, :])
                for slot in range(2):
                    po = slot * 64
                    ac = acc[po:po + 64, 2048 * seg + slot:2048 * (seg + 1):2]
                    nc.vector.tensor_add(ac, ac, o_ps[slot][0:64, :])
            else:
                # stage the two Z rows to SBUF on ACT first: the strided
                # zz scatters (16B-step, 2x cacheline penalty) then read
                # the staging copy instead of gating the o_ps banks that
                # the next job's AV accumulation needs to reuse.
                for slot in range(2):
                    zr = 64 * slot
                    nc.scalar.copy(zst[zr:zr + 1, :],
                                   o_ps[slot][64:65, :])
                for slot in range(2):
                    po = slot * 64
                    o0 = 2 * slot
                    for dd in range(2):
                        ac = acc[po:po + 64, o0 + dd::4]
                        nc.vector.scalar_tensor_tensor(
                            ac, o_ps[slot][0:64, :],
                            ws_sb[po:po + 64, dd:dd + 1],
                            ac, mybir.AluOpType.mult, mybir.AluOpType.add,
                        )
                for slot in range(2):
                    zr = 64 * slot
                    o0 = 2 * slot
                    for dd in range(2):
                        zc = zz[zr:zr + 1, o0 + dd::4]
                        nc.vector.scalar_tensor_tensor(
                            zc, zst[zr:zr + 1, :],
                            ws_sb[zr:zr + 1, dd:dd + 1],
                            zc, mybir.AluOpType.mult, mybir.AluOpType.add,
                        )

        zw = persist.tile([128, 64], F32R, tag="zw")
        rzp = persist.tile([2, 8 * PB], F32R, tag="rzp")

        def norm_prep(half):
            """Reciprocal pipeline for blocks [4*half, 4*half+4) — no PE
            instructions, safe to emit while jobs still run. Requires the
            zz columns of that half to be final."""
            p0 = 64 * half
            for i, zr in enumerate((0, 64)):
                nc.sync.dma_start(
                    zw[p0:p0 + 64, 32 * i:32 * i + 32].bitcast(F32),
                    zz[zr:zr + 1, 2048 * half:2048 * (half + 1)],
                )
            with nc.allow_low_precision(reason="tf32 norm"):
                nc.vector.reciprocal(zw[p0:p0 + 64, :], zw[p0:p0 + 64, :])
            for pb in range(4 * half, 4 * half + 4):
                nc.sync.dma_start(rzp[0:1, pb * PB:(pb + 1) * PB],
                                  zw[16 * pb:16 * pb + 16, 0:32])
                nc.sync.dma_start(rzp[1:2, pb * PB:(pb + 1) * PB],
                                  zw[16 * pb:16 * pb + 16, 32:64])

        def norm_apply(half):
            for pb in range(4 * half, 4 * half + 4):
                rb = psw.tile([128, 1024], F32, tag="w")
                nc.tensor.matmul(
                    rb[:, 0:PB], ind_sb[:],
                    rzp[:, pb * PB:(pb + 1) * PB], start=True, stop=True,
                )
                anorm = opool.tile([128, PB], BF16, tag="anorm")
                nc.vector.tensor_mul(
                    anorm[:], acc[:, pb * PB:(pb + 1) * PB], rb[:, 0:PB]
                )
                if stage >= 6:
                    # scalar queue: don't head-of-line block behind
                    # the rzp/zw gathers on the sync HWDGE ring
                    nc.scalar.dma_start(a2a_in[pb], anorm[:])

        # ================= emission order =================
        load_quarter("k", 0)
        load_quarter("q", 0)
        load_quarter("v", 0)
        load_quarter("k", 1)
        proj_quarter("k", 0)
        proj_quarter("q", 0)
        # tiny dummy collective: absorbs the ~30us first-collective setup
        # cost while the projections stream.
        if stage >= 6:
            for rr in range(8):
                nc.scalar.dma_start(a2a_warm_in[rr][0:1, 0:16],
                                    ind_sb[0:1, 0:16])
            nc.gpsimd.collective_compute(
                "AllToAll", mybir.AluOpType.bypass,
                replica_groups=[list(range(8))],
                ins=[a2a_warm_in[:]], outs=[a2a_warm_out[:]],
            )
        proj_quarter("v", 0)
        if dbg and stage <= 1:
            for qt in range(1, 4):
                for name in ("k", "q", "v"):
                    if (name, qt) not in xin_tiles:
                        load_quarter(name, qt)
                    proj_quarter(name, qt)
            nc.sync.dma_start(dbg_qt[:], QT[:])
            nc.sync.dma_start(dbg_kt[:], KT[:])
            nc.sync.dma_start(dbg_vt[:], VT[:])
            return
        load_quarter("q", 1)
        load_quarter("v", 1)
        f1 = [lambda n=n, p=p: proj_pb(n, p)
              for n in ("k", "q", "v") for p in (2, 3)]
        if stage >= 3:
            job(0, 0, fill=f1 if stage >= 4 else ())()
            if stage < 4:
                for f in f1:
                    f()
        else:
            for f in f1:
                f()
        load_quarter("k", 2)
        load_quarter("q", 2)
        load_quarter("v", 2)
        f2a = [lambda n=n, p=p: proj_pb(n, p)
               for n in ("k", "q") for p in (4, 5)]
        f2b = [lambda p=p: proj_pb("v", p) for p in (4, 5)]
        load_quarter("k", 3)
        if stage >= 4:
            job(0, 1, fill=f2a)()
            job(1, 0, fill=f2b)()
        else:
            for f in f2a + f2b:
                f()
        load_quarter("q", 3)
        load_quarter("v", 3)
        f3 = [lambda n=n, p=p: proj_pb(n, p)
              for n in ("k", "q", "v") for p in (6, 7)]
        if stage >= 4:
            job(0, 2, fill=f3)()
        else:
            for f in f3:
                f()
        b2_copies()
        if stage >= 6:
            nc.scalar.dma_start(wo_sb_0[:], wo[0])
            nc.scalar.dma_start(wo_sb_1[:], wo[1])
        if stage >= 4:
            # deferred merges are injected into the next job so its
            # transpose/scores stream isn't gated on the DVE backlog;
            # br0's first-touch copy still precedes br2's adds (m03
            # before m20 on the DVE stream). Each late job's V-prep is
            # emitted as PE filler inside the previous job so the
            # exp-bound stretches keep the tensor engine dense.
            vb20 = vpool.tile([128, 2 * 8 * VBW], BF16, tag="vb")
            m03 = job(0, 3, fill=(lambda: vprep(2, 0, vb20, act_b=False),))
            vb11 = vpool.tile([128, 2 * 8 * VBW], BF16, tag="vb")
            m20 = job(2, 0, inject=(m03,), vb=vb20,
                      fill=(lambda: vprep(1, 1, vb11, act_b=False),))
            inj = [m20]
            if stage >= 5:
                inj.append(lambda: norm_prep(0))
            m11 = job(1, 1, inject=inj, vb=vb11)
            if stage >= 5:
                norm_apply(0)   # cols 0-2047 already final before m11
            m11()

        if dbg:
            nc.sync.dma_start(dbg_qt[:], QT[:])
            nc.sync.dma_start(dbg_kt[:], KT[:])
            nc.sync.dma_start(dbg_vt[:], VT[:])
            nc.sync.dma_start(dbg_q2[:], QT2[:])
            if stage >= 3:
                nc.sync.dma_start(dbg_acc[:], acc[:])
                nc.sync.dma_start(dbg_zz[:], zz[:])
        if stage <= 4:
            return

        norm_prep(1)
        norm_apply(1)
        if dbg and stage == 5:
            nc.sync.dma_start(dbg_acc[:], acc[:])
        if stage <= 5:
            return

        # ---- AllToAll + output projection (ec-outer: consume each
        # gathered block as it lands) ----
        nc.gpsimd.collective_compute(
            "AllToAll", mybir.AluOpType.bypass,
            replica_groups=[list(range(8))],
            ins=[a2a_in[:]], outs=[a2a_out[:]],
        )
        for s in range(8):
            nc.sync.dma_start(mg[:, s * PB:(s + 1) * PB], a2a_out[s])
        wo_sb = [wo_sb_0, wo_sb_1]
        pt_w0 = psw.tile([128, 1024], F32, tag="w")
        pt_w1 = psw.tile([128, 1024], F32, tag="w")
        pt_o0 = pso.tile([128, 1024], F32, tag="o")
        pt_o1 = pso.tile([128, 1024], F32, tag="o")
        pts = [pt_w0, pt_w1, pt_o0, pt_o1]
        def drain_ob(ob):
            # alternate engines/queues so the 8-block drain runs two-wide
            osb = opool.tile([128, PB], F32, tag="osb")
            src_ap = pts[ob // 2][:, (ob % 2) * PB:(ob % 2 + 1) * PB]
            if ob % 2 == 0:
                nc.vector.tensor_scalar_add(osb[:], src_ap,
                                            bo_sb[:, ob:ob + 1])
                nc.sync.dma_start(outT[ob * 128:(ob + 1) * 128, :], osb[:])
            else:
                nc.scalar.add(osb[:], src_ap, bo_sb[:, ob:ob + 1])
                nc.scalar.dma_start(outT[ob * 128:(ob + 1) * 128, :], osb[:])

        for ec in range(KC):
            w_t = wo_sb[ec // 4]
            for ob in range(8):
                lhs = w_t[:, (ec % 4) * E + ob * 128:
                          (ec % 4) * E + (ob + 1) * 128]
                nc.tensor.matmul(
                    pts[ob // 2][:, (ob % 2) * PB:(ob % 2 + 1) * PB],
                    lhs, mg[:, ec * PB:(ec + 1) * PB],
                    start=(ec == 0), stop=(ec == KC - 1),
                )
        for ob in range(8):
            drain_ob(ob)

    with tile.TileContext(nc) as tc, contextlib.ExitStack() as ctx:
        with nc.allow_low_precision(reason="bf16 pipeline"):
            _emit(tc, ctx)

    nc.compile()
    return nc


_NC_CACHE = {}


def _get_nc(stage=6, dbg=False):
    key = (stage, dbg)
    if key not in _NC_CACHE:
        _NC_CACHE[key] = _build(stage, dbg)
    return _NC_CACHE[key]


def _bf16(a):
    return a.astype(mybir.dt.np(BF16))


def _prep_inputs(query, key, value, Wq, bq, Wk, bk, Wv, bv, Wo, bo):
    """Host-side sharding/layout prep. Returns in_maps for the 8 cores."""
    def pack_x(x):
        # (L, E) -> [pos-block, partition, kc*512] bf16
        xT = np.ascontiguousarray(x[0].T)            # (E, L)
        a = xT.reshape(KC, 128, 8, PB)               # (kc, p, pb, pos)
        a = a.transpose(2, 1, 0, 3).reshape(8, 128, KC * PB)
        return _bf16(np.ascontiguousarray(a))

    xq_p = pack_x(query)
    xk_p = pack_x(key)
    xv_p = pack_x(value)

    WqT = np.ascontiguousarray(Wq.T) * np.float32(0.125)
    WkT = np.ascontiguousarray(Wk.T)
    WvT = np.ascontiguousarray(Wv.T)

    def pack_w(WT, sel):
        # (E, 128-slice) -> [p, kc*128] bf16
        w = WT[:, sel].reshape(KC, 128, 128).transpose(1, 0, 2)
        return _bf16(np.ascontiguousarray(w.reshape(128, KC * 128)))

    # permuted Wo.T rows to match a2a feature order
    perm = np.concatenate(
        [np.r_[64 * s:64 * s + 64, 512 + 64 * s:512 + 64 * s + 64]
         for s in range(8)]
    )
    WoT = np.ascontiguousarray(Wo.T)[perm]  # (E e', E o)
    wo_pack = np.zeros((2, 128, 4 * E), np.float32)
    for ec in range(8):
        wo_pack[ec // 4, :, (ec % 4) * E:(ec % 4 + 1) * E] = \
            WoT[ec * 128:(ec + 1) * 128]
    wo_pack = _bf16(wo_pack)

    bo_eff = (bo + bv @ Wo.T).astype(np.float32)
    bo8 = bo_eff.reshape(8, 128).T.copy()  # [p, ob]

    # per-core offset indicators: slot A offset = c//4 in {0,1} on rows 0-63,
    # slot B offset = 2 + c//4 (encoded as its low bit) on rows 64-127.
    WS = np.zeros((8, 128, 2), np.float32)
    for c in range(8):
        d = c // 4
        WS[c, 0:64, d] = 1.0
        WS[c, 64:128, d] = 1.0

    IND = np.zeros((2, 128), np.float32)
    IND[0, 0:64] = 1.0
    IND[1, 64:128] = 1.0
    EYE = _bf16(np.eye(128, dtype=np.float32))
    ONES16 = _bf16(np.ones((128, 16), np.float32))

    in_maps = []
    for c in range(8):
        fa = np.r_[64 * c:64 * c + 64]
        fb = np.r_[512 + 64 * c:512 + 64 * c + 64]
        sel = np.concatenate([fa, fb])
        in_maps.append({
            "xq": xq_p, "xk": xk_p, "xv": xv_p,
            "wq": pack_w(WqT, sel),
            "wk": pack_w(WkT, sel),
            "wv": pack_w(WvT, sel),
            "wo": wo_pack,
            "bq": (bq[sel] * np.float32(0.125)).reshape(128, 1).astype(np.float32),
            "bk": bk[sel].reshape(128, 1).astype(np.float32),
            "bo8": bo8,
            "ind2": IND, "eyer": EYE, "ones16": ONES16,
            "wsel": WS[c],
        })
    return in_maps


def kernel(query, key, value, Wq, bq, Wk, bk, Wv, bv, Wo, bo,
           _trace=False, _result_holder=None, _stage=6, _dbg=False):
    args = [np.asarray(a, np.float32) for a in
            (query, key, value, Wq, bq, Wk, bk, Wv, bv, Wo, bo)]
    nc = _get_nc(_stage, _dbg)
    in_maps = _prep_inputs(*args)
    res = bass_utils.run_bass_kernel_spmd(
        nc, in_maps, core_ids=list(range(N_CORES)), trace=_trace
    )
    if _result_holder is not None:
        _result_holder.append(res)
    outT = np.zeros((E, L), np.float32)
    for c in range(N_CORES):
        outT[:, PB * c:PB * (c + 1)] = res.results[c]["outT"]
    return np.ascontiguousarray(outT.T).reshape(1, L, E)


# revision 7
# speedup vs baseline: 1.3241x; 1.3241x over previous
"""Dilated (LongNet-style) attention kernel for 8 TRN2 NeuronCores — v2.

Same head-sharded single-AllToAll strategy as v1, rebuilt around three
trace-driven fixes:
  - inputs stream as bf16 in 2MB quarter-chunks on HWDGE queues only
    (24MB instead of 48MB; ~400GB/s instead of ~190; gpsimd freed)
  - all on-chip activations (Q^T/K^T/V^T, e, vb) are bf16: halves SBUF
    and doubles most DVE copy throughput
  - emission keeps the PE stream dense (proj quarters and attention jobs
    interleaved without DMA waits between matmul bursts) so the HAM
    clock-gate stays at 8/8 instead of the 4/8 the v1 trace showed.
"""

import sys

if "/opt/trn_rl_repo" not in sys.path:
    sys.path.insert(0, "/opt/trn_rl_repo")

import numpy as np

import concourse.bacc as bacc
import concourse.bass as bass  # noqa: F401
import concourse.mybir as mybir
import concourse.tile as tile
from concourse import bass_utils

F32 = mybir.dt.float32
F32R = mybir.dt.float32r
BF16 = mybir.dt.bfloat16
AF = mybir.ActivationFunctionType

N_CORES = 8
E, L, H, D = 1024, 4096, 16, 64
KC = 8          # contraction chunks of 128 for the projections
PB = 512        # position block
NPB = L // PB   # 8
QTR = 1024      # dma quarter: positions per input-stream chunk
G = 1024        # compressed segment length (same for every branch)
VBW = 65        # V_both per-chunk width (64 feats + ones col)


def _build(stage=6, dbg=False):
    nc = bacc.Bacc("TRN2", target_bir_lowering=False, debug=False,
                   num_devices=N_CORES)

    # inputs: per-stream packed [pos-block, partition(=feat%128), kc*512pos]
    xq = nc.dram_tensor("xq", [8, 128, KC * PB], BF16, kind="ExternalInput")
    xk = nc.dram_tensor("xk", [8, 128, KC * PB], BF16, kind="ExternalInput")
    xv = nc.dram_tensor("xv", [8, 128, KC * PB], BF16, kind="ExternalInput")
    wq = nc.dram_tensor("wq", [128, KC * 128], BF16, kind="ExternalInput")
    wk = nc.dram_tensor("wk", [128, KC * 128], BF16, kind="ExternalInput")
    wv = nc.dram_tensor("wv", [128, KC * 128], BF16, kind="ExternalInput")
    wo = nc.dram_tensor("wo", [2, 128, 4 * E], BF16, kind="ExternalInput")
    bq = nc.dram_tensor("bq", [128, 1], F32, kind="ExternalInput")
    bk = nc.dram_tensor("bk", [128, 1], F32, kind="ExternalInput")
    bo8 = nc.dram_tensor("bo8", [128, 8], F32, kind="ExternalInput")
    ind2 = nc.dram_tensor("ind2", [2, 128], F32R, kind="ExternalInput")
    eyer = nc.dram_tensor("eyer", [128, 128], BF16, kind="ExternalInput")
    ones16 = nc.dram_tensor("ones16", [128, 16], BF16, kind="ExternalInput")
    wsel = nc.dram_tensor("wsel", [128, 2], F32, kind="ExternalInput")

    outT = nc.dram_tensor("outT", [E, PB], F32, kind="ExternalOutput")
    if dbg:
        dbg_qt = nc.dram_tensor("dbg_qt", [128, L], BF16, kind="ExternalOutput")
        dbg_kt = nc.dram_tensor("dbg_kt", [128, L], BF16, kind="ExternalOutput")
        dbg_vt = nc.dram_tensor("dbg_vt", [128, L], BF16, kind="ExternalOutput")
        dbg_q2 = nc.dram_tensor("dbg_q2", [128, G], BF16, kind="ExternalOutput")
        dbg_acc = nc.dram_tensor("dbg_acc", [128, L], F32, kind="ExternalOutput")
        dbg_zz = nc.dram_tensor("dbg_zz", [65, L], F32, kind="ExternalOutput")

    a2a_warm_in = nc.dram_tensor("a2a_warm_in", [8, 1, 64], F32R)
    a2a_warm_out = nc.dram_tensor("a2a_warm_out", [8, 1, 64], F32R)
    a2a_in = nc.dram_tensor("a2a_in", [8, 128, PB], BF16)
    a2a_out = nc.dram_tensor("a2a_out", [8, 128, PB], BF16)

    import contextlib

    def _emit(tc, ctx):
        pin = ctx.enter_context(tc.tile_pool(name="pin", bufs=6))
        persist = ctx.enter_context(tc.tile_pool(name="persist", bufs=1))
        vpool = ctx.enter_context(tc.tile_pool(name="vpool", bufs=3))
        epool = ctx.enter_context(tc.tile_pool(name="epool", bufs=4))
        opool = ctx.enter_context(tc.tile_pool(name="opool", bufs=3))
        psw = ctx.enter_context(tc.tile_pool(name="psw", bufs=2, space="PSUM"))
        pso = ctx.enter_context(tc.tile_pool(name="pso", bufs=2, space="PSUM"))

        # ---- small constants (scalar queue; inputs go on sync) ----
        wq_sb = persist.tile([128, KC * 128], BF16, tag="wq")
        wk_sb = persist.tile([128, KC * 128], BF16, tag="wk")
        wv_sb = persist.tile([128, KC * 128], BF16, tag="wv")
        nc.scalar.dma_start(wq_sb[:], wq[:])
        nc.scalar.dma_start(wk_sb[:], wk[:])
        nc.scalar.dma_start(wv_sb[:], wv[:])
        bq_sb = persist.tile([128, 1], F32, tag="bq")
        bk_sb = persist.tile([128, 1], F32, tag="bk")
        bo_sb = persist.tile([128, 8], F32, tag="bo")
        ind_sb = persist.tile([2, 128], F32R, tag="ind")
        eye_sb = persist.tile([128, 128], BF16, tag="eye")
        on_sb = persist.tile([128, 16], BF16, tag="on")
        ws_sb = persist.tile([128, 2], F32, tag="ws")
        nc.scalar.dma_start(bq_sb[:], bq[:])
        nc.scalar.dma_start(bk_sb[:], bk[:])
        nc.scalar.dma_start(bo_sb[:], bo8[:])
        nc.scalar.dma_start(ind_sb[:], ind2[:])
        nc.scalar.dma_start(eye_sb[:], eyer[:])
        nc.scalar.dma_start(on_sb[:], ones16[:])
        nc.scalar.dma_start(ws_sb[:], wsel[:])

        QT = persist.tile([128, L], BF16, tag="QT")
        KT = persist.tile([128, L], BF16, tag="KT")
        VT = persist.tile([128, L], BF16, tag="VT")
        QT2 = persist.tile([128, G], BF16, tag="QT2")
        KT2 = persist.tile([128, G], BF16, tag="KT2")
        VT2 = persist.tile([128, G], BF16, tag="VT2")
        acc = persist.tile([128, L], F32, tag="acc")
        zz = persist.tile([65, L], F32, tag="zz")
        zst = persist.tile([65, G], F32, tag="zst")  # rows 0/64 (lane-aligned)
        mg = persist.tile([128, 8 * PB], BF16, tag="mg")
        wo_sb_0 = persist.tile([128, 4 * E], BF16, tag="wo0")
        wo_sb_1 = persist.tile([128, 4 * E], BF16, tag="wo1")

        streams = {
            "k": (xk, wk_sb, KT, bk_sb),
            "v": (xv, wv_sb, VT, None),
            "q": (xq, wq_sb, QT, bq_sb),
        }

        xin_tiles = {}

        def load_pb(name, pb):
            x_d = streams[name][0]
            xin = pin.tile([128, KC * PB], BF16, tag="xin")
            nc.sync.dma_start(xin[:], x_d[pb])
            xin_tiles[(name, pb)] = xin

        def load_quarter(name, qt):
            load_pb(name, 2 * qt)
            load_pb(name, 2 * qt + 1)

        def proj_pb(name, pb):
            """Project one 1MB input position block."""
            x_d, w_t, dst, bias = streams[name]
            xin = xin_tiles.pop((name, pb))
            pt = psw.tile([128, 1024], F32, tag="w")
            for kc in range(KC):
                nc.tensor.matmul(
                    pt[:, 0:PB],
                    w_t[:, kc * 128:(kc + 1) * 128],
                    xin[:, kc * PB:(kc + 1) * PB],
                    start=(kc == 0), stop=(kc == KC - 1),
                )
            dslice = dst[:, pb * PB:(pb + 1) * PB]
            if name == "k":
                if pb < 2:
                    nc.scalar.add(dslice, pt[:, 0:PB], bk_sb[:, 0:1])
                else:
                    nc.vector.tensor_scalar_add(dslice, pt[:, 0:PB],
                                                bk_sb[:, 0:1])
            elif name == "q":
                nc.vector.tensor_scalar_add(dslice, pt[:, 0:PB], bias[:])
            else:
                nc.vector.tensor_copy(dslice, pt[:, 0:PB])

        def proj_quarter(name, qt):
            proj_pb(name, 2 * qt)
            proj_pb(name, 2 * qt + 1)

        def b2_copies():
            # slot A picks dense offset 0 or 1, slot B picks 2 or 3, via
            # 0/1 indicators in ws_sb (core-uniform instruction stream).
            for src, dst, eng in ((QT, QT2, nc.vector), (KT, KT2, nc.vector),
                                  (VT, VT2, nc.vector)):
                for slot in range(2):
                    p0, p1 = 64 * slot, 64 * slot + 64
                    o0 = 2 * slot
                    eng.tensor_scalar_mul(
                        dst[p0:p1, :], src[p0:p1, o0::4], ws_sb[p0:p1, 0:1]
                    )
                    eng.scalar_tensor_tensor(
                        dst[p0:p1, :], src[p0:p1, o0 + 1::4],
                        ws_sb[p0:p1, 1:2], dst[p0:p1, :],
                        mybir.AluOpType.mult, mybir.AluOpType.add,
                    )

        def kq_slice(br, seg, slot, t, lo, size):
            if br == 0:
                base = 1024 * seg + lo
                return t[slot * 64:(slot + 1) * 64, base:base + size]
            if br == 1:
                base = 2048 * seg + 2 * lo + slot
                return t[slot * 64:(slot + 1) * 64,
                         base:base + 2 * size - slot:2]
            return t[slot * 64:(slot + 1) * 64, lo:lo + size]

        def mm_ranges(jc):
            if jc <= 3:
                return [(128 * jc, 512 - 128 * jc), (512, 512)]
            return [(128 * jc, 1024 - 128 * jc)]

        def vprep(br, seg, vb, act_b=True):
            """V_both prep (PE transpose -> vb, bf16). Copies split DVE
            (slot A) / ACT (slot B: only when act_b — during the late,
            exp-saturated jobs everything stays on DVE)."""
            nc.vector.tensor_copy(vb[:, 64::VBW], on_sb[:])
            for jc in range(8):
                if br == 0:
                    tp = psw.tile([128, 1024], BF16, tag="w")
                    src = VT[:, 1024 * seg + 128 * jc:1024 * seg + 128 * (jc + 1)]
                    nc.tensor.transpose(tp[:, 0:128], src, eye_sb[:])
                    nc.vector.tensor_copy(
                        vb[:, jc * VBW:jc * VBW + 64], tp[:, 0:64])
                    bcp = nc.scalar.copy if act_b else nc.vector.tensor_copy
                    bcp(vb[:, 8 * VBW + jc * VBW:8 * VBW + jc * VBW + 64],
                        tp[:, 64:128])
                else:
                    for slot in range(2):
                        tp = psw.tile([128, 1024], BF16, tag="w")
                        if br == 1:
                            base = 2048 * seg + 256 * jc + slot
                            src = VT[slot * 64:(slot + 1) * 64,
                                     base:base + 256 - slot:2]
                        else:
                            src = VT2[slot * 64:(slot + 1) * 64,
                                      128 * jc:128 * (jc + 1)]
                        nc.tensor.transpose(
                            tp[:, 0:64], src,
                            eye_sb[slot * 64:(slot + 1) * 64,
                                   slot * 64:(slot + 1) * 64],
                        )
                        if slot == 0:
                            nc.vector.tensor_copy(
                                vb[:, jc * VBW:jc * VBW + 64], tp[:, 0:64])
                        elif act_b:
                            nc.scalar.copy(
                                vb[:, 8 * VBW + jc * VBW:
                                   8 * VBW + jc * VBW + 64],
                                tp[:, 0:64])
                        else:
                            nc.vector.tensor_copy(
                                vb[:, 8 * VBW + jc * VBW:
                                   8 * VBW + jc * VBW + 64],
                                tp[:, 0:64])

        def job(br, seg, inject=(), fill=(), vb=None):
            """One (branch, segment) attention job, software-pipelined:
            scores(jc+1) is emitted before AV(jc) so the PE never waits on
            the exp chain. `inject` closures (deferred merges / norm prep)
            are emitted between the first scores and the steady loop.
            `fill` closures (proj position-blocks) are spread one per jc
            iteration as PE filler so the engine stays dense (HAM-warm)
            while the ACT exp chain runs."""
            fill = list(fill)
            kt_src = KT2 if br == 2 else KT
            qt_src = QT2 if br == 2 else QT

            o_ps_a = pso.tile([128, 1024], F32, tag="o")
            o_ps_b = pso.tile([128, 1024], F32, tag="o")
            o_ps = [o_ps_a, o_ps_b]

            def scores(jc):
                s_ps_a = psw.tile([128, 1024], F32, tag="w")
                s_ps_b = psw.tile([128, 1024], F32, tag="w")
                s_ps = [s_ps_a, s_ps_b]
                for slot in range(2):
                    for lo, size in mm_ranges(jc):
                        lhs = kq_slice(br, seg, slot, kt_src, 128 * jc, 128)
                        rhs = kq_slice(br, seg, slot, qt_src, lo, size)
                        nc.tensor.matmul(
                            s_ps[slot][:, lo:lo + size], lhs, rhs,
                            start=True, stop=True,
                            tile_position=(slot * 64, 0),
                        )
                return s_ps

            s_cur = scores(0)
            if vb is None:
                vb = vpool.tile([128, 2 * 8 * VBW], BF16, tag="vb")
                vprep(br, seg, vb)
            for cl in inject:
                cl()
            for jc in range(8):
                e_t = [None, None]
                for slot in range(2):
                    c0 = 128 * jc
                    e = epool.tile([128, 1024], BF16, tag="e")  # noqa
                    nc.scalar.activation(
                        e[:, c0:1024], s_cur[slot][:, c0:1024], AF.Exp
                    )
                    nc.gpsimd.affine_select(
                        e[:, c0:c0 + 128], e[:, c0:c0 + 128],
                        pattern=[[1, 128]],
                        compare_op=mybir.AluOpType.is_ge,
                        fill=0.0, base=0, channel_multiplier=-1,
                    )
                    e_t[slot] = e
                if jc < 7:
                    if fill:
                        fill.pop(0)()
                    s_cur = scores(jc + 1)
                for slot in range(2):
                    for lo, size in mm_ranges(jc):
                        nc.tensor.matmul(
                            o_ps[slot][0:VBW, lo:lo + size],
                            vb[:, slot * 8 * VBW + jc * VBW:
                               slot * 8 * VBW + (jc + 1) * VBW],
                            e_t[slot][:, lo:lo + size],
                            start=(jc == 0), stop=(jc == 7),
                        )

            def merge():
                _merge(br, seg, o_ps)
            return merge

        def _merge(br, seg, o_ps):
            # merge into acc / zz (slot B copies on ACT to offload DVE).
            # zz updates go first so the normalization reciprocal chain
            # can start as early as possible.
            if br == 0:
                sl_ = slice(1024 * seg, 1024 * (seg + 1))
                nc.vector.tensor_copy(zz[0:1, sl_], o_ps[0][64:65, :])
                nc.scalar.copy(zz[64:65, sl_], o_ps[1][64:65, :])
                nc.vector.tensor_copy(acc[0:64, sl_], o_ps[0][0:64, :])
                nc.scalar.copy(acc[64:128, sl_], o_ps[1][0:64, :])
            elif br == 1:
                for slot in range(2):
                    zc = zz[64 * slot:64 * slot + 1,
                            2048 * seg + slot:2048 * (seg + 1):2]
                    nc.vector.tensor_add(zc, zc, o_ps[slot][64:65, :])
                for slot in range(2):
                    po = slot * 64
                    ac = acc[po:po + 64, 2048 * seg + slot:2048 * (seg + 1):2]
                    nc.vector.tensor_add(ac, ac, o_ps[slot][0:64, :])
            else:
                # stage the two Z rows to SBUF on ACT first: the strided
                # zz scatters (16B-step, 2x cacheline penalty) then read
                # the staging copy instead of gating the o_ps banks that
                # the next job's AV accumulation needs to reuse.
                for slot in range(2):
                    zr = 64 * slot
                    nc.scalar.copy(zst[zr:zr + 1, :],
                                   o_ps[slot][64:65, :])
                for slot in range(2):
                    po = slot * 64
                    o0 = 2 * slot
                    for dd in range(2):
                        ac = acc[po:po + 64, o0 + dd::4]
                        nc.vector.scalar_tensor_tensor(
                            ac, o_ps[slot][0:64, :],
                            ws_sb[po:po + 64, dd:dd + 1],
                            ac, mybir.AluOpType.mult, mybir.AluOpType.add,
                        )
                for slot in range(2):
                    zr = 64 * slot
                    o0 = 2 * slot
                    for dd in range(2):
                        zc = zz[zr:zr + 1, o0 + dd::4]
                        nc.vector.scalar_tensor_tensor(
                            zc, zst[zr:zr + 1, :],
                            ws_sb[zr:zr + 1, dd:dd + 1],
                            zc, mybir.AluOpType.mult, mybir.AluOpType.add,
                        )

        zw = persist.tile([128, 64], F32R, tag="zw")
        rzp = persist.tile([2, 8 * PB], F32R, tag="rzp")

        def norm_prep(half):
            """Reciprocal pipeline for blocks [4*half, 4*half+4) — no PE
            instructions, safe to emit while jobs still run. Requires the
            zz columns of that half to be final."""
            p0 = 64 * half
            for i, zr in enumerate((0, 64)):
                nc.sync.dma_start(
                    zw[p0:p0 + 64, 32 * i:32 * i + 32].bitcast(F32),
                    zz[zr:zr + 1, 2048 * half:2048 * (half + 1)],
                )
            with nc.allow_low_precision(reason="tf32 norm"):
                nc.vector.reciprocal(zw[p0:p0 + 64, :], zw[p0:p0 + 64, :])
            for pb in range(4 * half, 4 * half + 4):
                nc.sync.dma_start(rzp[0:1, pb * PB:(pb + 1) * PB],
                                  zw[16 * pb:16 * pb + 16, 0:32])
                nc.sync.dma_start(rzp[1:2, pb * PB:(pb + 1) * PB],
                                  zw[16 * pb:16 * pb + 16, 32:64])

        def norm_apply(half):
            for pb in range(4 * half, 4 * half + 4):
                rb = psw.tile([128, 1024], F32, tag="w")
                nc.tensor.matmul(
                    rb[:, 0:PB], ind_sb[:],
                    rzp[:, pb * PB:(pb + 1) * PB], start=True, stop=True,
                )
                anorm = opool.tile([128, PB], BF16, tag="anorm")
                nc.vector.tensor_mul(
                    anorm[:], acc[:, pb * PB:(pb + 1) * PB], rb[:, 0:PB]
                )
                if stage >= 6:
                    # scalar queue: don't head-of-line block behind
                    # the rzp/zw gathers on the sync HWDGE ring
                    nc.scalar.dma_start(a2a_in[pb], anorm[:])

        # ================= emission order =================
        load_quarter("k", 0)
        load_quarter("q", 0)
        load_quarter("v", 0)
        load_quarter("k", 1)
        proj_quarter("k", 0)
        proj_quarter("q", 0)
        # tiny dummy collective: absorbs the ~30us first-collective setup
        # cost while the projections stream.
        if stage >= 6:
            for rr in range(8):
                nc.scalar.dma_start(a2a_warm_in[rr][0:1, 0:16],
                                    ind_sb[0:1, 0:16])
            nc.gpsimd.collective_compute(
                "AllToAll", mybir.AluOpType.bypass,
                replica_groups=[list(range(8))],
                ins=[a2a_warm_in[:]], outs=[a2a_warm_out[:]],
            )
        proj_quarter("v", 0)
        if dbg and stage <= 1:
            for qt in range(1, 4):
                for name in ("k", "q", "v"):
                    if (name, qt) not in xin_tiles:
                        load_quarter(name, qt)
                    proj_quarter(name, qt)
            nc.sync.dma_start(dbg_qt[:], QT[:])
            nc.sync.dma_start(dbg_kt[:], KT[:])
            nc.sync.dma_start(dbg_vt[:], VT[:])
            return
        load_quarter("q", 1)
        load_quarter("v", 1)
        f1 = [lambda n=n, p=p: proj_pb(n, p)
              for n in ("k", "q", "v") for p in (2, 3)]
        if stage >= 3:
            job(0, 0, fill=f1 if stage >= 4 else ())()
            if stage < 4:
                for f in f1:
                    f()
        else:
            for f in f1:
                f()
        load_quarter("k", 2)
        load_quarter("q", 2)
        load_quarter("v", 2)
        f2a = [lambda n=n, p=p: proj_pb(n, p)
               for n in ("k", "q") for p in (4, 5)]
        f2b = [lambda p=p: proj_pb("v", p) for p in (4, 5)]
        load_quarter("k", 3)
        if stage >= 4:
            job(0, 1, fill=f2a)()
            job(1, 0, fill=f2b)()
        else:
            for f in f2a + f2b:
                f()
        load_quarter("q", 3)
        load_quarter("v", 3)
        f3 = [lambda n=n, p=p: proj_pb(n, p)
              for n in ("k", "q", "v") for p in (6, 7)]
        if stage >= 4:
            job(0, 2, fill=f3)()
        else:
            for f in f3:
                f()
        b2_copies()
        if stage >= 6:
            nc.scalar.dma_start(wo_sb_0[:], wo[0])
            nc.scalar.dma_start(wo_sb_1[:], wo[1])
        if stage >= 4:
            # deferred merges are injected into the next job so its
            # transpose/scores stream isn't gated on the DVE backlog;
            # br0's first-touch copy still precedes br2's adds (m03
            # before m20 on the DVE stream). Each late job's V-prep is
            # emitted as PE filler inside the previous job so the
            # exp-bound stretches keep the tensor engine dense.
            vb20 = vpool.tile([128, 2 * 8 * VBW], BF16, tag="vb")
            m03 = job(0, 3, fill=(lambda: vprep(2, 0, vb20, act_b=False),))
            vb11 = vpool.tile([128, 2 * 8 * VBW], BF16, tag="vb")
            m20 = job(2, 0, inject=(m03,), vb=vb20,
                      fill=(lambda: vprep(1, 1, vb11, act_b=False),))
            inj = [m20]
            if stage >= 5:
                inj.append(lambda: norm_prep(0))
            m11 = job(1, 1, inject=inj, vb=vb11)
            if stage >= 5:
                norm_apply(0)   # cols 0-2047 already final before m11
            m11()

        if dbg:
            nc.sync.dma_start(dbg_qt[:], QT[:])
            nc.sync.dma_start(dbg_kt[:], KT[:])
            nc.sync.dma_start(dbg_vt[:], VT[:])
            nc.sync.dma_start(dbg_q2[:], QT2[:])
            if stage >= 3:
                nc.sync.dma_start(dbg_acc[:], acc[:])
                nc.sync.dma_start(dbg_zz[:], zz[:])
        if stage <= 4:
            return

        norm_prep(1)
        norm_apply(1)
        if dbg and stage == 5:
            nc.sync.dma_start(dbg_acc[:], acc[:])
        if stage <= 5:
            return

        # ---- AllToAll + output projection (ec-outer: consume each
        # gathered block as it lands) ----
        nc.gpsimd.collective_compute(
            "AllToAll", mybir.AluOpType.bypass,
            replica_groups=[list(range(8))],
            ins=[a2a_in[:]], outs=[a2a_out[:]],
        )
        for s in range(8):
            nc.sync.dma_start(mg[:, s * PB:(s + 1) * PB], a2a_out[s])
        wo_sb = [wo_sb_0, wo_sb_1]
        pt_w0 = psw.tile([128, 1024], F32, tag="w")
        pt_w1 = psw.tile([128, 1024], F32, tag="w")
        pt_o0 = pso.tile([128, 1024], F32, tag="o")
        pt_o1 = pso.tile([128, 1024], F32, tag="o")
        pts = [pt_w0, pt_w1, pt_o0, pt_o1]
        def drain_ob(ob):
            osb = opool.tile([128, PB], F32, tag="osb")
            nc.vector.tensor_scalar_add(
                osb[:], pts[ob // 2][:, (ob % 2) * PB:(ob % 2 + 1) * PB],
                bo_sb[:, ob:ob + 1]
            )
            nc.sync.dma_start(outT[ob * 128:(ob + 1) * 128, :], osb[:])

        for ec in range(KC):
            w_t = wo_sb[ec // 4]
            for ob in range(8):
                lhs = w_t[:, (ec % 4) * E + ob * 128:
                          (ec % 4) * E + (ob + 1) * 128]
                nc.tensor.matmul(
                    pts[ob // 2][:, (ob % 2) * PB:(ob % 2 + 1) * PB],
                    lhs, mg[:, ec * PB:(ec + 1) * PB],
                    start=(ec == 0), stop=(ec == KC - 1),
                )
        for ob in range(8):
            drain_ob(ob)

    with tile.TileContext(nc) as tc, contextlib.ExitStack() as ctx:
        with nc.allow_low_precision(reason="bf16 pipeline"):
            _emit(tc, ctx)

    nc.compile()
    return nc


_NC_CACHE = {}


def _get_nc(stage=6, dbg=False):
    key = (stage, dbg)
    if key not in _NC_CACHE:
        _NC_CACHE[key] = _build(stage, dbg)
    return _NC_CACHE[key]


def _bf16(a):
    return a.astype(mybir.dt.np(BF16))


def _prep_inputs(query, key, value, Wq, bq, Wk, bk, Wv, bv, Wo, bo):
    """Host-side sharding/layout prep. Returns in_maps for the 8 cores."""
    def pack_x(x):
        # (L, E) -> [pos-block, partition, kc*512] bf16
        xT = np.ascontiguousarray(x[0].T)            # (E, L)
        a = xT.reshape(KC, 128, 8, PB)               # (kc, p, pb, pos)
        a = a.transpose(2, 1, 0, 3).reshape(8, 128, KC * PB)
        return _bf16(np.ascontiguousarray(a))

    xq_p = pack_x(query)
    xk_p = pack_x(key)
    xv_p = pack_x(value)

    WqT = np.ascontiguousarray(Wq.T) * np.float32(0.125)
    WkT = np.ascontiguousarray(Wk.T)
    WvT = np.ascontiguousarray(Wv.T)

    def pack_w(WT, sel):
        # (E, 128-slice) -> [p, kc*128] bf16
        w = WT[:, sel].reshape(KC, 128, 128).transpose(1, 0, 2)
        return _bf16(np.ascontiguousarray(w.reshape(128, KC * 128)))

    # permuted Wo.T rows to match a2a feature order
    perm = np.concatenate(
        [np.r_[64 * s:64 * s + 64, 512 + 64 * s:512 + 64 * s + 64]
         for s in range(8)]
    )
    WoT = np.ascontiguousarray(Wo.T)[perm]  # (E e', E o)
    wo_pack = np.zeros((2, 128, 4 * E), np.float32)
    for ec in range(8):
        wo_pack[ec // 4, :, (ec % 4) * E:(ec % 4 + 1) * E] = \
            WoT[ec * 128:(ec + 1) * 128]
    wo_pack = _bf16(wo_pack)

    bo_eff = (bo + bv @ Wo.T).astype(np.float32)
    bo8 = bo_eff.reshape(8, 128).T.copy()  # [p, ob]

    # per-core offset indicators: slot A offset = c//4 in {0,1} on rows 0-63,
    # slot B offset = 2 + c//4 (encoded as its low bit) on rows 64-127.
    WS = np.zeros((8, 128, 2), np.float32)
    for c in range(8):
        d = c // 4
        WS[c, 0:64, d] = 1.0
        WS[c, 64:128, d] = 1.0

    IND = np.zeros((2, 128), np.float32)
    IND[0, 0:64] = 1.0
    IND[1, 64:128] = 1.0
    EYE = _bf16(np.eye(128, dtype=np.float32))
    ONES16 = _bf16(np.ones((128, 16), np.float32))

    in_maps = []
    for c in range(8):
        fa = np.r_[64 * c:64 * c + 64]
        fb = np.r_[512 + 64 * c:512 + 64 * c + 64]
        sel = np.concatenate([fa, fb])
        in_maps.append({
            "xq": xq_p, "xk": xk_p, "xv": xv_p,
            "wq": pack_w(WqT, sel),
            "wk": pack_w(WkT, sel),
            "wv": pack_w(WvT, sel),
            "wo": wo_pack,
            "bq": (bq[sel] * np.float32(0.125)).reshape(128, 1).astype(np.float32),
            "bk": bk[sel].reshape(128, 1).astype(np.float32),
            "bo8": bo8,
            "ind2": IND, "eyer": EYE, "ones16": ONES16,
            "wsel": WS[c],
        })
    return in_maps


def kernel(query, key, value, Wq, bq, Wk, bk, Wv, bv, Wo, bo,
           _trace=False, _result_holder=None, _stage=6, _dbg=False):
    args = [np.asarray(a, np.float32) for a in
            (query, key, value, Wq, bq, Wk, bk, Wv, bv, Wo, bo)]
    nc = _get_nc(_stage, _dbg)
    in_maps = _prep_inputs(*args)
    res = bass_utils.run_bass_kernel_spmd(
        nc, in_maps, core_ids=list(range(N_CORES)), trace=_trace
    )
    if _result_holder is not None:
        _result_holder.append(res)
    outT = np.zeros((E, L), np.float32)
    for c in range(N_CORES):
        outT[:, PB * c:PB * (c + 1)] = res.results[c]["outT"]
    return np.ascontiguousarray(outT.T).reshape(1, L, E)
